# revision 11
# baseline (speedup 1.0000x reference)
"""Trainium2 Bass kernel for ConvolutionalSelfAttention.

Math (per batch image, fp32):
  X [256, 64] pixels.  For each 3x3 window n (196 of them) and local slot k
  (9), the reference softmax-attends over the 247 pixels outside window n
  with logits TEMP*cos(x_g, x_{pix(n,k)}), weights s_g = x_g @ Wg + bg, and
  aggregates the window pixels with the resulting per-slot weights.

  Key factorization: all needed cosine sims live in one 256x256 gram
  E = exp(TEMP * Xn @ Xn.T); window/global masking is linear, so
      D[p, n] = sum_g maskg[g, n] * E[g, p]          (denominator)
      N[p, n] = sum_g maskg[g, n] * s'_g * E[g, p]   (numerator)
      A[p, n] = maskl[p, n] * N[p, n] / D[p, n]
      out[n, c] = sum_p A[p, n] * X[p, c]
  -> everything is dense matmuls + one exp, no per-window gathers.

Sharding: data-parallel over batch; 32 images / 8 cores = 4 images per core.
"""

import sys
import numpy as np

sys.path.insert(0, "/opt/trn_rl_repo")

from contextlib import ExitStack

import concourse.bass as bass
import concourse.bacc as bacc
import concourse.tile as tile
from concourse import mybir
from concourse.bass_utils import run_bass_kernel_spmd

H = 16
W = 16
C = 64
K = 3
B = 32
CH = H - K + 1
CW = W - K + 1
NC = CH * CW          # 196
HW = H * W            # 256
TEMP = 10.0
NCORES = 8
BPC = B // NCORES     # 4 images per core
P = 128

F32 = mybir.dt.float32
AF = mybir.ActivationFunctionType
ALU = mybir.AluOpType


def _masks():
    maskl = np.zeros((HW, NC), np.float32)
    for i in range(CH):
        for j in range(CW):
            n = i * CW + j
            m = np.zeros((H, W), bool)
            m[i:i + K, j:j + K] = True
            maskl[m.reshape(-1), n] = 1.0
    return maskl, (1.0 - maskl).astype(np.float32)


MASKL, MASKG = _masks()
IDENT = np.eye(P if (P:=128) else 128, dtype=np.float32)


def _bcast_ap(ap, parts):
    """[*dims] -> [parts, *dims] with partition stride 0 (DMA broadcast)."""
    return bass.AP(tensor=ap.tensor, offset=ap.offset, ap=[[0, parts]] + list(ap.ap))


def build_bass():
    nc = bacc.Bacc("TRN2", target_bir_lowering=False, debug=False)

    x = nc.declare_dram_parameter("x", [BPC, HW, C], F32, isOutput=False)
    wg = nc.declare_dram_parameter("wg", [C, 1], F32, isOutput=False)
    bg = nc.declare_dram_parameter("bg", [1], F32, isOutput=False)
    mgd = nc.declare_dram_parameter("maskg", [HW, NC], F32, isOutput=False)
    mld = nc.declare_dram_parameter("maskl", [HW, NC], F32, isOutput=False)
    idd = nc.declare_dram_parameter("ident", [P, P], F32, isOutput=False)
    y = nc.declare_dram_parameter("y", [BPC, NC, C], F32, isOutput=True)

    with ExitStack() as ctx:
        tc = ctx.enter_context(tile.TileContext(nc))
        consts = ctx.enter_context(tc.tile_pool(name="consts", bufs=1))
        sb = ctx.enter_context(tc.tile_pool(name="sb", bufs=2))
        pt_pool = ctx.enter_context(tc.tile_pool(name="pt", bufs=1, space="PSUM"))
        pg_pool = ctx.enter_context(tc.tile_pool(name="pg", bufs=1, space="PSUM"))
        pnd_pool = ctx.enter_context(tc.tile_pool(name="pnd", bufs=1, space="PSUM"))

        ident = consts.tile([P, P], F32, tag="ident")
        nc.sync.dma_start(out=ident, in_=idd[:, :])

        wb = consts.tile([P, C], F32, tag="wb")
        nc.sync.dma_start(out=wb, in_=_bcast_ap(wg[:, 0], P))
        bgb = consts.tile([P, 1], F32, tag="bgb")
        nc.sync.dma_start(out=bgb, in_=_bcast_ap(bg[:], P))

        mg = []
        ml = []
        for t in range(2):
            mgt = consts.tile([P, NC], F32, tag=f"mg{t}")
            nc.sync.dma_start(out=mgt, in_=mgd[t * P:(t + 1) * P, :])
            mg.append(mgt)
            mlt = consts.tile([P, NC], F32, tag=f"ml{t}")
            nc.sync.dma_start(out=mlt, in_=mld[t * P:(t + 1) * P, :])
            ml.append(mlt)

        for b in range(BPC):
            # ---- load image ----
            xt = []
            for t in range(2):
                xtt = sb.tile([P, C], F32, tag=f"x{t}")
                nc.sync.dma_start(out=xtt, in_=x[b, t * P:(t + 1) * P, :])
                xt.append(xtt)

            # ---- per-pixel: sum-of-squares, 1/||x||, s = x@Wg + bg ----
            xn = []
            sp = []
            for t in range(2):
                scr = sb.tile([P, C], F32, tag=f"scr{t}")
                nc.vector.tensor_mul(out=scr, in0=xt[t], in1=xt[t])
                ss = sb.tile([P, 1], F32, tag=f"ss{t}")
                nc.vector.reduce_sum(out=ss, in_=scr, axis=mybir.AxisListType.X)
                scr2 = sb.tile([P, C], F32, tag=f"scr2{t}")
                nc.vector.tensor_mul(out=scr2, in0=xt[t], in1=wb)
                s0 = sb.tile([P, 1], F32, tag=f"s0{t}")
                nc.vector.reduce_sum(out=s0, in_=scr2, axis=mybir.AxisListType.X)
                spt = sb.tile([P, 1], F32, tag=f"sp{t}")
                nc.vector.tensor_scalar_add(out=spt, in0=s0, scalar1=bgb[:, 0:1])
                sp.append(spt)
                # rn = exp(-0.5*ln(ss)) = 1/sqrt(ss)   (stays in ln/exp table set)
                u = sb.tile([P, 1], F32, tag=f"u{t}")
                nc.scalar.activation(out=u, in_=ss, func=AF.Ln)
                rn = sb.tile([P, 1], F32, tag=f"rn{t}")
                nc.scalar.activation(out=rn, in_=u, func=AF.Exp, scale=-0.5)
                xnt = sb.tile([P, C], F32, tag=f"xn{t}")
                nc.vector.tensor_scalar_mul(out=xnt, in0=xt[t], scalar1=rn)
                xn.append(xnt)

            # ---- transpose normalized pixels -> XnT [64, 256] ----
            xnT = sb.tile([C, HW], F32, tag="xnT")
            for t in range(2):
                tp = pt_pool.tile([C, P], F32, tag=f"tp{t}")
                nc.tensor.transpose(out=tp, in_=xn[t], identity=ident)
                nc.vector.tensor_copy(out=xnT[:, t * P:(t + 1) * P], in_=tp)

            # ---- gram + exp: E = exp(TEMP * XnT.T @ XnT)  [256, 256] ----
            e = []
            for t in range(2):
                g = pg_pool.tile([P, HW], F32, tag=f"g{t}")
                nc.tensor.matmul(
                    out=g, lhsT=xnT[:, t * P:(t + 1) * P], rhs=xnT,
                    start=True, stop=True)
                et = sb.tile([P, HW], F32, tag=f"e{t}")
                nc.scalar.activation(out=et, in_=g, func=AF.Exp, scale=TEMP)
                e.append(et)

            # ---- masked numerator / denominator sums over g ----
            ms = []
            for t in range(2):
                mst = sb.tile([P, NC], F32, tag=f"ms{t}")
                nc.vector.tensor_scalar_mul(out=mst, in0=mg[t], scalar1=sp[t])
                ms.append(mst)

            a = []
            for pti in range(2):
                psl = slice(pti * P, (pti + 1) * P)
                d_ps = pnd_pool.tile([P, NC], F32, tag=f"d{pti}")
                nc.tensor.matmul(out=d_ps, lhsT=e[0][:, psl], rhs=mg[0],
                                 start=True, stop=False)
                nc.tensor.matmul(out=d_ps, lhsT=e[1][:, psl], rhs=mg[1],
                                 start=False, stop=True)
                n_ps = pnd_pool.tile([P, NC], F32, tag=f"n{pti}")
                nc.tensor.matmul(out=n_ps, lhsT=e[0][:, psl], rhs=ms[0],
                                 start=True, stop=False)
                nc.tensor.matmul(out=n_ps, lhsT=e[1][:, psl], rhs=ms[1],
                                 start=False, stop=True)

                # 1/D = exp(-ln(D)); A = maskl * N * (1/D)
                u2 = sb.tile([P, NC], F32, tag=f"u2{pti}")
                nc.scalar.activation(out=u2, in_=d_ps, func=AF.Ln)
                rd = sb.tile([P, NC], F32, tag=f"rd{pti}")
                nc.scalar.activation(out=rd, in_=u2, func=AF.Exp, scale=-1.0)
                a1 = sb.tile([P, NC], F32, tag=f"a1{pti}")
                nc.vector.tensor_mul(out=a1, in0=n_ps, in1=rd)
                a2 = sb.tile([P, NC], F32, tag=f"a2{pti}")
                nc.vector.tensor_mul(out=a2, in0=a1, in1=ml[pti])
                a.append(a2)

            # ---- out[n, c] = sum_p A[p, n] X[p, c] ----
            for nt, (n0, nsz) in enumerate(((0, P), (P, NC - P))):
                o = pg_pool.tile([P, C], F32, tag=f"g{nt}")
                nc.tensor.matmul(out=o[:nsz, :], lhsT=a[0][:, n0:n0 + nsz],
                                 rhs=xt[0], start=True, stop=False)
                nc.tensor.matmul(out=o[:nsz, :], lhsT=a[1][:, n0:n0 + nsz],
                                 rhs=xt[1], start=False, stop=True)
                osb = sb.tile([P, C], F32, tag=f"osb{nt}")
                nc.vector.tensor_copy(out=osb[:nsz, :], in_=o[:nsz, :])
                nc.sync.dma_start(out=y[b, n0:n0 + nsz, :], in_=osb[:nsz, :])

    nc.compile()
    return nc


_NC_CACHE = None


def _get_nc():
    global _NC_CACHE
    if _NC_CACHE is None:
        _NC_CACHE = build_bass()
    return _NC_CACHE


def kernel(batch: np.ndarray, Wg: np.ndarray, bg: np.ndarray) -> np.ndarray:
    X = np.ascontiguousarray(np.asarray(batch, np.float32).reshape(B, HW, C))
    wgf = np.ascontiguousarray(np.asarray(Wg, np.float32))
    bgf = np.ascontiguousarray(np.asarray(bg, np.float32))

    nc = _get_nc()
    in_maps = [
        {
            "x": X[c * BPC:(c + 1) * BPC],
            "wg": wgf,
            "bg": bgf,
            "maskg": MASKG,
            "maskl": MASKL,
            "ident": IDENT,
        }
        for c in range(NCORES)
    ]
    res = run_bass_kernel_spmd(nc, in_maps, list(range(NCORES)))
    out = np.concatenate([np.asarray(res.results[c]["y"]) for c in range(NCORES)], 0)
    return out.reshape(B, CH, CW, C).astype(np.float32)
